# revision 2
# baseline (speedup 1.0000x reference)
"""Trainium2 Bass kernel for nn_CausalBankModel (decay-bank LM head), v2.

Strategy (8 NeuronCores, vocab-tensor-parallel):
  - Every core computes the shared trunk (mode projection, decay-bank scan,
    both hidden layers) redundantly (~25% of per-core PE time).
  - The two big [2048,1024]@[1024,4000] readout matmuls are sharded over the
    vocab dim: core c owns vocab columns [c*4000, (c+1)*4000).
  - Token tiles are processed in 3 groups (6/6/4).  After a group finishes,
    its per-position partial stats (sum/sumsq/max over the local vocab slice,
    both branches) go through ONE AllGather; the gate + mix for that group
    runs overlapped with the next group's matmuls.  Logits are staged per
    (ti, branch) in SBUF, spilled to DRAM as single 1MB transfers, and read
    back for the mix (prefetched; only the last group's AllGather + mix DVE
    is exposed at the tail).

Layouts on device (partition dim first):
  - xT     : [128(d), dh, b, 7+S] f32r transposed embeddings, 7 zero pad cols
  - statesT: [128(m), mt, b, S] f32r decay-bank states (tensor_tensor_scan)
  - hT/h2T : [128(hidden), kt, B*S] bf16, lhsT of the big matmuls
  - wt     : [128(k), kt*512+c] bf16 pre-swizzled weight chunks (host prep)
  - logits : [128(token), vocab-slice] bf16 staging tiles
"""

import os
import sys

import numpy as np

for _p in ("/opt/trn_rl_repo", "/opt/pypackages"):
    if _p not in sys.path and os.path.isdir(_p):
        sys.path.append(_p)

import ml_dtypes  # noqa: E402

from concourse import bacc, bass, tile  # noqa: E402
from concourse import mybir  # noqa: E402
from concourse.bass_utils import run_bass_kernel_spmd  # noqa: E402

F32 = mybir.dt.float32
F32R = mybir.dt.float32r
BF16 = mybir.dt.bfloat16
ALU = mybir.AluOpType
ACTF = mybir.ActivationFunctionType

V = 32000
D = 256
M = 256
W = 8
HL = 1024  # hidden width of both readout MLPs
B = 2
S = 1024
BS = B * S            # 2048 tokens
NCORE = 8
VSH = V // NCORE      # 4000 vocab cols per core
NVC = 8               # vocab chunks per core
CW = [512] * 7 + [VSH - 7 * 512]          # chunk widths (last = 416)
COFF = [512 * i for i in range(NVC)]      # chunk offsets
NT = BS // 128        # 16 token tiles
SP = S + W - 1        # 1031, padded time length
GROUPS = [(0, 6), (6, 6), (12, 4)]        # (first ti, n ti)

LAST_RESULT = None


def build(nc, with_vocab_bias=True):
    din = {}

    def inp(name, shape, dt):
        din[name] = nc.dram_tensor(name, list(shape), dt, kind="ExternalInput")
        return din[name]

    xt_d = inp("xt", [128, 2 * B * SP], F32R)
    inproj_d = inp("inproj", [D, M], F32R)
    decb_d = inp("decb", [M, 512], F32)
    w1_d = inp("w1", [M + D, HL], F32R)
    b1r_d = inp("b1r", [128, HL // 128], F32)
    lw1_d = inp("lw1", [W * D, HL], F32R)
    lb1r_d = inp("lb1r", [128, HL // 128], F32)
    # pre-swizzled big weights: [vc, p, kt*512+c] = W[kt*128+p, vc*512+c]
    w2s_d = inp("w2s", [NVC, 128, NVC * 512], BF16)
    lw2s_d = inp("lw2s", [NVC, 128, NVC * 512], BF16)
    b2_d = inp("b2", [1, NVC * 512], BF16)
    lb2_d = inp("lb2", [1, NVC * 512], BF16)
    ones_d = inp("ones", [1, 128], BF16)
    gwb_d = inp("gwb", [128, 6], F32)
    gbb_d = inp("gbb", [128, 1], F32)

    out_d = nc.dram_tensor("out", [BS, VSH], BF16, kind="ExternalOutput")

    with tile.TileContext(nc) as tc:
        with (
            tc.tile_pool(name="cst", bufs=1) as cst,
            tc.tile_pool(name="ps", bufs=8, space=bass.MemorySpace.PSUM) as psp,
            tc.tile_pool(name="dram", bufs=1, space="DRAM") as drp,
        ):
            gwb_sb = cst.tile([128, 6], F32)
            nc.sync.dma_start(gwb_sb[:], gwb_d[:, :])
            gbb_sb = cst.tile([128, 1], F32)
            nc.sync.dma_start(gbb_sb[:], gbb_d[:, :])
            g_sb = cst.tile([128, NT], F32)
            if with_vocab_bias:
                ones_sb = cst.tile([1, 128], BF16)
                nc.sync.dma_start(ones_sb[:], ones_d[:, :])
                b2_sb = cst.tile([1, NVC * 512], BF16)
                nc.sync.dma_start(b2_sb[:], b2_d[:, :])
                lb2_sb = cst.tile([1, NVC * 512], BF16)
                nc.sync.dma_start(lb2_sb[:], lb2_d[:, :])
            else:
                b2_sb = lb2_sb = None

            # DRAM spill for per-(ti,branch) logits + collective buffers
            lin_dr = drp.tile([NT, 128, VSH], BF16)
            loc_dr = drp.tile([NT, 128, VSH], BF16)
            ag_in = {}
            ag_out = {}
            for gi, (_, gn) in enumerate(GROUPS[:-1]):
                ag_in[gi] = drp.tile([128, 6 * gn], F32, name=f"agin{gi}")
                ag_out[gi] = drp.tile([NCORE * 128, 6 * gn], F32,
                                      name=f"agout{gi}")
            for ti in range(GROUPS[-1][0], GROUPS[-1][0] + GROUPS[-1][1]):
                ag_in[f"t{ti}"] = drp.tile([128, 6], F32, name=f"agint{ti}")
                ag_out[f"t{ti}"] = drp.tile([NCORE * 128, 6], F32,
                                            name=f"agoutt{ti}")

            with tc.tile_pool(name="ph", bufs=1) as php:  # trunk out, spans all
                hT = php.tile([128, 8, BS], BF16)
                h2T = php.tile([128, 8, BS], BF16)

                # ---------------- trunk ----------------
                with tc.tile_pool(name="pa", bufs=1) as pap:
                    xT = pap.tile([128, 2, B, SP], F32R)

                    with tc.tile_pool(name="plw", bufs=1) as plw:
                        with tc.tile_pool(name="psc", bufs=1) as psc:
                            # small trunk tiles FIRST so the u-matmuls can
                            # start ~10us in; then xT (batch-major), then the
                            # big lw1 block which is only needed ~25us in.
                            inproj_sb = psc.tile([128, 2, M], F32R)
                            for kt in range(2):
                                nc.sync.dma_start(
                                    inproj_sb[:, kt, :],
                                    inproj_d[kt * 128:(kt + 1) * 128, :])
                            decb_sb = psc.tile([128, 2, 512], F32)
                            for mt in range(2):
                                nc.sync.dma_start(decb_sb[:, mt, :],
                                                  decb_d[mt * 128:(mt + 1) * 128, :])
                            b1r_sb = psc.tile([128, 8], F32)
                            nc.sync.dma_start(b1r_sb[:], b1r_d[:, :])
                            for b in range(B):
                                for dh in range(2):
                                    nc.sync.dma_start(
                                        xT[:, dh, b, :],
                                        xt_d[:, (dh * B + b) * SP:
                                             (dh * B + b + 1) * SP])
                            w1_sb = psc.tile([128, 4, HL], F32R)
                            for kt in range(4):
                                nc.sync.dma_start(w1_sb[:, kt, :],
                                                  w1_d[kt * 128:(kt + 1) * 128, :])
                            lw1_sb = plw.tile([128, 16, HL], F32R)
                            for kt in range(16):
                                nc.sync.dma_start(lw1_sb[:, kt, :],
                                                  lw1_d[kt * 128:(kt + 1) * 128, :])
                            lb1r_sb = plw.tile([128, 8], F32)
                            nc.sync.dma_start(lb1r_sb[:], lb1r_d[:, :])
                            statesT = psc.tile([128, 2, B, S], F32R)

                            # modes + decay-bank scan (DVE; overlaps A2b below)
                            for b in range(B):
                                for mt in range(2):
                                    for hf in range(2):
                                        ps = psp.tile([128, 512], F32, name="ps", tag="ps")
                                        for kt in range(2):
                                            nc.tensor.matmul(
                                                ps[:],
                                                inproj_sb[:, kt,
                                                          mt * 128:(mt + 1) * 128],
                                                xT[:, kt, b,
                                                   W - 1 + hf * 512:
                                                   W - 1 + hf * 512 + 512],
                                                start=(kt == 0), stop=(kt == 1),
                                            )
                                        init = (0.0 if hf == 0 else
                                                statesT[:, mt, b,
                                                        hf * 512 - 1:hf * 512])
                                        nc.vector.tensor_tensor_scan(
                                            statesT[:, mt, b,
                                                    hf * 512:hf * 512 + 512],
                                            decb_sb[:, mt, :], ps[:], init,
                                            ALU.mult, ALU.add,
                                        )

                            # local-window hidden (needs only xT + lw1)
                            for hl in range(8):
                                for ch in range(4):
                                    b, hf = ch // 2, ch % 2
                                    ps = psp.tile([128, 512], F32, name="ps", tag="ps")
                                    for ki in range(16):
                                        w, dh = ki // 2, ki % 2
                                        rhs = xT[:, dh, b,
                                                 hf * 512 + w:hf * 512 + w + 512]
                                        nc.tensor.matmul(
                                            ps[:],
                                            lw1_sb[:, ki, hl * 128:(hl + 1) * 128],
                                            rhs, start=(ki == 0), stop=(ki == 15),
                                        )
                                    nc.scalar.activation(
                                        h2T[:, hl, ch * 512:(ch + 1) * 512], ps[:],
                                        ACTF.Relu, bias=lb1r_sb[:, hl:hl + 1])

                            # linear-readout hidden (needs statesT + xT + w1)
                            for hl in range(8):
                                for ch in range(4):
                                    b, hf = ch // 2, ch % 2
                                    ps = psp.tile([128, 512], F32, name="ps", tag="ps")
                                    for kt in range(4):
                                        if kt < 2:
                                            rhs = statesT[:, kt, b,
                                                          hf * 512:hf * 512 + 512]
                                        else:
                                            rhs = xT[:, kt - 2, b,
                                                     W - 1 + hf * 512:
                                                     W - 1 + hf * 512 + 512]
                                        nc.tensor.matmul(
                                            ps[:],
                                            w1_sb[:, kt, hl * 128:(hl + 1) * 128],
                                            rhs, start=(kt == 0), stop=(kt == 3),
                                        )
                                    nc.scalar.activation(
                                        hT[:, hl, ch * 512:(ch + 1) * 512], ps[:],
                                        ACTF.Relu, bias=b1r_sb[:, hl:hl + 1])

                # ---------------- vocab-sharded readout ----------------
                with (
                    tc.tile_pool(name="wst", bufs=2) as wst,
                    tc.tile_pool(name="lt", bufs=8) as ltp,
                    tc.tile_pool(name="sqp", bufs=2) as sqp,
                    tc.tile_pool(name="rawst", bufs=2) as rawp,
                    tc.tile_pool(name="mixr", bufs=3) as mxp,
                    tc.tile_pool(name="gtmp", bufs=2) as gtp,
                    tc.tile_pool(name="wres", bufs=8) as wresp,
                ):
                    branches = [(hT, w2s_d, b2_sb, lin_dr),
                                (h2T, lw2s_d, lb2_sb, loc_dr)]

                    def mix_ti(ti):
                        """out[ti] = g*lin + (1-g)*loc from the DRAM spill.
                        SWDGE readback in halves so the HWDGE weight stream
                        is never blocked behind 1MB transfers."""
                        for off in (0, VSH // 2):
                            w = VSH // 2
                            mlt = mxp.tile([128, VSH // 2], BF16, name="mlt",
                                           tag="mlt")
                            nc.gpsimd.dma_start(mlt[:],
                                                lin_dr[ti, :, off:off + w])
                            mct = mxp.tile([128, VSH // 2], BF16, name="mct",
                                           tag="mct")
                            nc.gpsimd.dma_start(mct[:],
                                                loc_dr[ti, :, off:off + w])
                            # in-place: mlt <- mlt-mct ; mlt <- mlt*g + mct
                            nc.vector.tensor_tensor(mlt[:], mlt[:], mct[:],
                                                    ALU.subtract)
                            nc.vector.scalar_tensor_tensor(
                                mlt[:], mlt[:], g_sb[:, ti:ti + 1], mct[:],
                                ALU.mult, ALU.add)
                            nc.gpsimd.dma_start(
                                out_d[ti * 128:(ti + 1) * 128, off:off + w],
                                mlt[:])

                    def fold_and_gate(key, t0, gn, st):
                        """AllGather packed stats st [128,6*gn], fold across
                        cores, compute gate -> g_sb[:, t0:t0+gn]."""
                        nc.sync.dma_start(ag_in[key][:, :], st[:])
                        nc.gpsimd.collective_compute(
                            "AllGather", ALU.bypass,
                            replica_groups=[list(range(NCORE))],
                            ins=[ag_in[key].opt()], outs=[ag_out[key].opt()])
                        agg = gtp.tile([128, NCORE, 6 * gn], F32,
                                       name="agg", tag="agg")
                        for r in range(NCORE):
                            nc.sync.dma_start(
                                agg[:, r, :],
                                ag_out[key][r * 128:(r + 1) * 128, :])
                        # tree-fold: adds on cols [0:4gn], max on [4gn:6gn]
                        for half in (4, 2, 1):
                            nc.vector.tensor_tensor(
                                agg[:, 0:half, 0:4 * gn],
                                agg[:, 0:half, 0:4 * gn],
                                agg[:, half:2 * half, 0:4 * gn], ALU.add)
                            nc.vector.tensor_tensor(
                                agg[:, 0:half, 4 * gn:6 * gn],
                                agg[:, 0:half, 4 * gn:6 * gn],
                                agg[:, half:2 * half, 4 * gn:6 * gn], ALU.max)

                        invV = 1.0 / float(V)
                        feats = []
                        for br in range(2):
                            mean = gtp.tile([128, gn], F32, name=f"mean{br}",
                                            tag=f"mean{br}")
                            nc.vector.tensor_scalar_mul(
                                mean[:],
                                agg[:, 0, 2 * br * gn:(2 * br + 1) * gn], invV)
                            ms = gtp.tile([128, gn], F32, name=f"ms{br}",
                                          tag=f"ms{br}")
                            nc.vector.tensor_scalar_mul(
                                ms[:], agg[:, 0, (2 * br + 1) * gn:
                                           (2 * br + 2) * gn], invV)
                            msq = gtp.tile([128, gn], F32, name=f"msq{br}",
                                           tag=f"msq{br}")
                            nc.vector.tensor_tensor(msq[:], mean[:], mean[:],
                                                    ALU.mult)
                            nc.vector.tensor_tensor(msq[:], ms[:], msq[:],
                                                    ALU.subtract)
                            nc.vector.tensor_scalar_max(msq[:], msq[:], 0.0)
                            std = gtp.tile([128, gn], F32, name=f"std{br}",
                                           tag=f"std{br}")
                            nc.scalar.activation(std[:], msq[:], ACTF.Sqrt)
                            feats.extend([
                                mean[:],
                                agg[:, 0, (4 + br) * gn:(5 + br) * gn],
                                std[:]])

                        acc = gtp.tile([128, gn], F32, name="acc", tag="acc")
                        nc.vector.tensor_scalar(acc[:], feats[0],
                                                gwb_sb[:, 0:1], None, ALU.mult)
                        for k in range(1, 6):
                            acc2 = gtp.tile([128, gn], F32, name=f"acc{k}",
                                            tag=f"acc{k}")
                            nc.vector.scalar_tensor_tensor(
                                acc2[:], feats[k], gwb_sb[:, k:k + 1], acc[:],
                                ALU.mult, ALU.add)
                            acc = acc2
                        nc.scalar.activation(g_sb[:, t0:t0 + gn], acc[:],
                                             ACTF.Sigmoid, bias=gbb_sb[:, 0:1])

                    def chunk(hsrc, wtile, ti, vc, sraw, sti, sp_dr, bias_sb):
                        """One [128,cw] logit chunk: matmuls, stats, spill."""
                        cw, co = CW[vc], COFF[vc]
                        ps = psp.tile([128, cw], F32, name="ps", tag="ps")
                        if with_vocab_bias:
                            nc.tensor.matmul(
                                ps[:], ones_sb[:, :],
                                bias_sb[:, vc * 512:vc * 512 + cw],
                                start=True, stop=False)
                        for kt in range(NVC):
                            nc.tensor.matmul(
                                ps[:],
                                hsrc[:, kt, ti * 128:(ti + 1) * 128],
                                wtile[:, kt * 512:kt * 512 + cw],
                                start=(kt == 0 and not with_vocab_bias),
                                stop=(kt == NVC - 1),
                            )
                        lt = ltp.tile([128, 512], BF16, name="lt", tag="lt")
                        nc.scalar.activation(
                            lt[:, 0:cw], ps[:], ACTF.Copy,
                            accum_out=sraw[0][:, sti, vc:vc + 1])
                        sq = sqp.tile([128, 512], BF16, name="sq", tag="sq")
                        nc.scalar.activation(
                            sq[:, 0:cw], lt[:, 0:cw], ACTF.Square,
                            accum_out=sraw[1][:, sti, vc:vc + 1])
                        nc.vector.tensor_reduce(
                            sraw[2][:, sti, vc:vc + 1], lt[:, 0:cw],
                            mybir.AxisListType.X, ALU.max)
                        nc.sync.dma_start(
                            sp_dr[ti, :, co:co + cw], lt[:, 0:cw])

                    STATS = ("sum", "sq", "mx")

                    # ---- groups 0..n-2: vc-outer streaming, group AG ----
                    for gi, (t0, gn) in enumerate(GROUPS[:-1]):
                        raw = []  # per branch: (ssum, ssq, smax)
                        for br in range(2):
                            raw.append(tuple(
                                rawp.tile([128, gn, NVC], F32,
                                          name=f"r{st}{br}", tag=f"r{st}{br}")
                                for st in STATS))
                        for vc in range(NVC):
                            wts = []
                            for br in range(2):
                                wt = wst.tile([128, NVC * 512], BF16,
                                              name=f"wt{br}", tag=f"wt{br}")
                                nc.sync.dma_start(wt[:],
                                                  branches[br][1][vc, :, :])
                                wts.append(wt)
                            for ti in range(t0, t0 + gn):
                                for br, (hsrc, wd, bias_sb,
                                         sp_dr) in enumerate(branches):
                                    chunk(hsrc, wts[br], ti, vc, raw[br],
                                          ti - t0, sp_dr, bias_sb)

                        st = gtp.tile([128, 6 * gn], F32, name="st", tag="st")
                        # layout: [sum_l | sq_l | sum_c | sq_c | mx_l | mx_c]
                        order = [raw[0][0], raw[0][1], raw[1][0], raw[1][1],
                                 raw[0][2], raw[1][2]]
                        for si, rt in enumerate(order):
                            nc.vector.tensor_reduce(
                                st[:, si * gn:(si + 1) * gn], rt[:],
                                mybir.AxisListType.X,
                                ALU.max if si >= 4 else ALU.add)
                        fold_and_gate(gi, t0, gn, st)
                        for ti in range(t0, t0 + gn):
                            mix_ti(ti)

                    # ---- last group: lin streamed, loc ti-outer with ----
                    # ---- resident weights + per-ti AG/gate/mix        ----
                    t0, gn = GROUPS[-1]
                    hsrc0, wd0, bias0, spd0 = branches[0]
                    hsrc1, wd1, bias1, spd1 = branches[1]
                    raw_lin = tuple(
                        rawp.tile([128, gn, NVC], F32,
                                  name=f"r{st}L", tag=f"r{st}0")
                        for st in STATS)
                    for vc in range(NVC):
                        wt = wst.tile([128, NVC * 512], BF16,
                                      name="wt0", tag="wt0")
                        nc.sync.dma_start(wt[:], wd0[vc, :, :])
                        for ti in range(t0, t0 + gn):
                            chunk(hsrc0, wt, ti, vc, raw_lin, ti - t0,
                                  spd0, bias0)
                    # lw2 for the last group goes fully resident (loaded
                    # during the lin pass above; separate pool, elastic)
                    wres_tiles = []
                    for vc in range(NVC):
                        wr = wresp.tile([128, NVC * 512], BF16,
                                        name="wr", tag="wr")
                        nc.sync.dma_start(wr[:], wd1[vc, :, :])
                        wres_tiles.append(wr)
                    for ti in range(t0, t0 + gn):
                        raw_loc = tuple(
                            rawp.tile([128, 1, NVC], F32,
                                      name=f"r{st}T", tag=f"r{st}1")
                            for st in STATS)
                        for vc in range(NVC):
                            chunk(hsrc1, wres_tiles[vc], ti, vc, raw_loc, 0,
                                  spd1, bias1)
                        st = gtp.tile([128, 6], F32, name="stt", tag="stt")
                        order = [raw_lin[0][:, ti - t0, :],
                                 raw_lin[1][:, ti - t0, :],
                                 raw_loc[0][:, 0, :], raw_loc[1][:, 0, :],
                                 raw_lin[2][:, ti - t0, :],
                                 raw_loc[2][:, 0, :]]
                        for si, rt in enumerate(order):
                            nc.vector.tensor_reduce(
                                st[:, si:si + 1], rt,
                                mybir.AxisListType.X,
                                ALU.max if si >= 4 else ALU.add)
                        fold_and_gate(f"t{ti}", ti, 1, st)
                        mix_ti(ti)

    nc.compile()
    return din, out_d


_CACHED = {}


def _get_program(with_vocab_bias):
    if with_vocab_bias not in _CACHED:
        nc = bacc.Bacc("TRN2", target_bir_lowering=False, debug=False,
                       num_devices=NCORE)
        build(nc, with_vocab_bias=with_vocab_bias)
        _CACHED[with_vocab_bias] = nc
    return _CACHED[with_vocab_bias]


def _prep_inputs(tokens, emb, in_proj, decays, w1, b1, w2, b2,
                 lw1, lb1, lw2, lb2, gate_w, gate_b):
    tokens = np.asarray(tokens).astype(np.int64).reshape(-1)  # [2048]
    emb = np.asarray(emb, np.float32)
    in_proj = np.asarray(in_proj, np.float32)
    decays = np.asarray(decays, np.float32)
    w1 = np.asarray(w1, np.float32)
    b1 = np.asarray(b1, np.float32)
    lw1 = np.asarray(lw1, np.float32)
    lb1 = np.asarray(lb1, np.float32)
    w2 = np.asarray(w2, np.float32)
    b2 = np.asarray(b2, np.float32)
    lw2 = np.asarray(lw2, np.float32)
    lb2 = np.asarray(lb2, np.float32)
    gate_w = np.asarray(gate_w, np.float32).reshape(6)
    gate_b = np.asarray(gate_b, np.float32).reshape(1)

    # host-side embedding gather + transpose into the device xT layout:
    # xt[d%128, (d//128, b)] at time col 7+s  ==  emb[tokens[b*S+s], d]
    x = emb[tokens].reshape(B, S, D)                     # [2, 1024, 256]
    xt = np.zeros((128, 2, B, SP), np.float32)
    for dh in range(2):
        for b in range(B):
            xt[:, dh, b, W - 1:] = x[b, :, dh * 128:(dh + 1) * 128].T
    xt = np.ascontiguousarray(xt.reshape(128, 2 * B * SP))

    shared = {
        "xt": xt,
        "inproj": in_proj,
        "decb": np.ascontiguousarray(np.broadcast_to(decays[:, None], (M, 512))),
        "w1": w1,
        "b1r": np.ascontiguousarray(b1.reshape(8, 128).T),
        "lw1": lw1,
        "lb1r": np.ascontiguousarray(lb1.reshape(8, 128).T),
        "ones": np.ones((1, 128), ml_dtypes.bfloat16),
        "gwb": np.ascontiguousarray(np.broadcast_to(gate_w[None, :], (128, 6))),
        "gbb": np.full((128, 1), gate_b[0], np.float32),
    }

    def swz(wfull, sl):
        """[1024, vocab-slice] -> [vc, p, kt*512+c] swizzled bf16."""
        wc = np.zeros((HL, NVC * 512), np.float32)
        wc[:, :VSH] = wfull[:, sl]
        return np.ascontiguousarray(
            wc.reshape(NVC, 128, NVC, 512).transpose(2, 1, 0, 3)
            .reshape(NVC, 128, NVC * 512).astype(ml_dtypes.bfloat16))

    in_maps = []
    for c in range(NCORE):
        sl = slice(c * VSH, (c + 1) * VSH)
        b2c = np.zeros((1, NVC * 512), ml_dtypes.bfloat16)
        b2c[0, :VSH] = b2[sl].astype(ml_dtypes.bfloat16)
        lb2c = np.zeros((1, NVC * 512), ml_dtypes.bfloat16)
        lb2c[0, :VSH] = lb2[sl].astype(ml_dtypes.bfloat16)
        m = dict(shared)
        m.update({"w2s": swz(w2, sl), "lw2s": swz(lw2, sl),
                  "b2": b2c, "lb2": lb2c})
        in_maps.append(m)
    return in_maps


def kernel(**inputs):
    global LAST_RESULT
    with_vocab_bias = bool(np.any(np.asarray(inputs["b2"]))
                           or np.any(np.asarray(inputs["lb2"])))
    nc = _get_program(with_vocab_bias)
    in_maps = _prep_inputs(**inputs)
    res = run_bass_kernel_spmd(nc, in_maps, list(range(NCORE)))
    LAST_RESULT = res
    full = np.empty((B, S, V), np.float32)
    for c in range(NCORE):
        full[:, :, c * VSH:(c + 1) * VSH] = (
            res.results[c]["out"].astype(np.float32).reshape(B, S, VSH))
    return full
